# revision 1
# baseline (speedup 1.0000x reference)
"""BernNet node-classification kernel for 8 Trainium2 NeuronCores.

Math: the reference computes out = log_softmax(sum_j T_j C(K,j)/2^K (I+A)^{K-j}(I-A)^j z)
with A = D^{-1/2} S D^{-1/2} (S = adjacency scatter by dst, D = src-degree).
Expanded in the monomial basis, out = log_softmax(sum_m c_m A^m z), needing only
K SpMVs. In scaled space t_m = D^{-1/2} A^m z the recurrence is t_{m+1} = D^{-1} S t_m
(plain scatter-sum + per-node 1/deg scale) and out = D^{1/2} sum_m c_m t_m at deg>0
rows (deg==0 rows fall back to c_0 z).

Device mapping per core (edges sharded by dst, 12500 nodes/shard):
 - MLP on PE in fp16 (x^T pre-transposed host-side).
 - Per iteration: dma_gather (80B payload rows at 256B stride; int16 indices per
   32768-row window, one SWDGE queue per window) fetches t[src] for edge slots,
   grouped 4 slots/lane by dst node; a DVE pair-tree reduces each lane; a
   dma_scatter_add (fp16 CCE add) accumulates lane partials into an HBM shard
   table, partitioned by lane-rank so indices are unique per instruction; then
   scale by 1/deg and AllGather to rebuild the replicated node table.
"""
import math
import os
import sys

sys.path.insert(0, '/opt/trn_rl_repo')
import numpy as np

from concourse import bass, bacc, mybir, tile
from concourse.bass_utils import run_bass_kernel_spmd

N_NODES = 100000
N_FEATS = 512
HIDDEN = 256
N_CLASSES = 40
K = 10
NCORES = 8
SHARD = N_NODES // NCORES          # 12500
RPS = 12800                        # table rows per shard (128*100)
NTILES = RPS // 128                # 100
TROWS = RPS * NCORES               # 102400
STEP = 128                         # fp16 elems per table row (256B stride)
FEAT = N_CLASSES                   # 40
G = 4                              # slots per lane
WIN = 32768                        # int16 index window
NWIN = 4
CHUNK = 4096                       # gather slots per instruction
MAXRANK = 12                       # max lanes per (node, window)

F16 = mybir.dt.float16
F32 = mybir.dt.float32
I16 = mybir.dt.int16

LAST_EXEC_NS = None


def _emit_dma_gather(eng, out_ap, in_ap, idxs_ap, num_idxs, elem_size, elem_step,
                     queue_num=0):
    """Like nc.gpsimd.dma_gather but allows a payload not divisible by 256B
    (only the row stride must be a 256B multiple)."""
    assert idxs_ap.dtype == I16
    assert num_idxs % 128 == 0
    assert in_ap.ap[-1][1] == out_ap.ap[-1][1] == elem_size
    assert in_ap.ap[0][0] == elem_step
    stride_bytes = elem_step * mybir.dt.size(in_ap.dtype)
    assert stride_bytes % 256 == 0
    _in_ap = eng.lower_ap_dma(in_ap, for_custom_bir_dma=True)
    _idxs_ap = eng.lower_ap(idxs_ap)
    _out_ap = eng.lower_ap(out_ap)
    return eng.add_instruction(
        mybir.InstDMAGatherAnt(
            name=eng.bass.get_next_instruction_name(),
            ins=[*_in_ap, _idxs_ap, eng.lower_val_access(eng.to_reg(num_idxs))],
            outs=[_out_ap],
            transpose=False,
            num_idxs=num_idxs,
            elem_size=elem_size,
            stride_bytes_256=stride_bytes // 256,
            gen_mode=0,
            single_packet=True,
            queue_num=queue_num,
            sbuf_tokens_per_rank=0,
            sbuf_free_dim_per_rank=0,
            sbuf_free_dim_pad_per_rank=0,
            sbuf_byte_offset=0,
        ))


def _host_prep(edge_index, temp):
    src = np.asarray(edge_index[0], dtype=np.int64)
    dst = np.asarray(edge_index[1], dtype=np.int64)

    deg = np.bincount(src, minlength=N_NODES).astype(np.float64)

    # monomial coefficients c_m of sum_j relu(T_j) C(K,j)/2^K (1+x)^{K-j}(1-x)^j
    T = np.maximum(np.asarray(temp, dtype=np.float64), 0.0)
    c = np.zeros(K + 1)
    for j in range(K + 1):
        pj = np.array([1.0])
        for _ in range(K - j):
            pj = np.convolve(pj, [1.0, 1.0])
        for _ in range(j):
            pj = np.convolve(pj, [1.0, -1.0])
        c += T[j] * (math.comb(K, j) / 2.0 ** K) * pj

    g_row = (src // SHARD) * RPS + (src % SHARD)
    g_win = g_row // WIN
    dst_shard = dst // SHARD
    dst_local = dst % SHARD

    lane_cnt_max = np.zeros((NWIN, MAXRANK), dtype=np.int64)
    core_data = []
    for cj in range(NCORES):
        wins = []
        csel = dst_shard == cj
        for w in range(NWIN):
            sel = csel & (g_win == w)
            n_l = dst_local[sel]
            s_row = (g_row[sel] - w * WIN).astype(np.int64)
            order = np.argsort(n_l, kind='stable')
            n_l = n_l[order]
            s_row = s_row[order]
            d = np.bincount(n_l, minlength=SHARD)
            lanes_n = (d + G - 1) // G
            for k in range(MAXRANK):
                lane_cnt_max[w, k] = max(lane_cnt_max[w, k], int((lanes_n > k).sum()))
            assert lanes_n.max(initial=0) <= MAXRANK
            wins.append((s_row, d, lanes_n))
        core_data.append(wins)

    c4_rank = (lane_cnt_max + 127) // 128          # [NWIN, MAXRANK]
    nc4_data = int(c4_rank.sum(axis=1).max())
    slots = ((nc4_data * 128 * G + CHUNK - 1) // CHUNK) * CHUNK
    nc4 = slots // (128 * G)                       # uniform columns per window
    return deg, c, core_data, c4_rank, nc4, slots


def _build_core_arrays(wins, c4_rank, nc4, slots, zero_row_local):
    """Per-core gather/scatter int16 index planes (partition-overlaid by window)."""
    trash = RPS - 2
    gplane = np.full((128, slots // 16), -1, dtype=np.int16)
    sfree = int(c4_rank.max(axis=0).sum()) * 8     # free cols of scatter plane
    rank_off = np.zeros((NWIN, MAXRANK + 1), dtype=np.int64)
    for w in range(NWIN):
        rank_off[w, 1:] = np.cumsum(c4_rank[w])
    splane = np.full((128, sfree), -1, dtype=np.int16)
    for w in range(NWIN):
        s_row, d, lanes_n = wins[w]
        ga = np.full(nc4 * 128 * G, int(zero_row_local[w]), dtype=np.int64)
        off = np.zeros(SHARD + 1, dtype=np.int64)
        off[1:] = np.cumsum(d)
        sfree_off = 0
        for k in range(MAXRANK):
            nck = int(c4_rank[w][k])
            if nck == 0:
                continue
            nk = np.nonzero(lanes_n > k)[0]
            sa = np.full(nck * 128, trash, dtype=np.int64)
            if len(nk):
                ordinal = np.arange(len(nk))
                c4_l = ordinal // 128
                p = ordinal % 128
                sa[c4_l * 128 + p] = nk
                lane_c4 = rank_off[w, k] + c4_l
                for s in range(G):
                    eidx = off[nk] + G * k + s
                    valid = eidx < off[nk] + d[nk]
                    pos = ((lane_c4 * G + s) * 128 + p)[valid]
                    ga[pos] = s_row[eidx[valid]]
            wr = sa.astype(np.int16).reshape(nck * 8, 16).T      # [16, nck*8]
            splane[32 * w:32 * w + 16, sfree_off:sfree_off + nck * 8] = wr
            splane[32 * w + 16:32 * w + 32, sfree_off:sfree_off + nck * 8] = wr
            sfree_off += nck * 8
        # trailing -1 trim of pure-pad chunk tails
        ga16 = ga.astype(np.int16)
        data_end = int(rank_off[w, MAXRANK]) * 128 * G
        for ci in range(slots // CHUNK):
            lo, hi = ci * CHUNK, (ci + 1) * CHUNK
            if lo >= data_end:
                ga16[lo:hi] = -1
            elif hi > data_end:
                ga16[data_end:hi] = -1
        gw = ga16.reshape(slots // 16, 16).T                     # [16, slots/16]
        gplane[32 * w:32 * w + 16, :] = gw
        gplane[32 * w + 16:32 * w + 32, :] = gw
    return gplane, splane, rank_off, sfree


def kernel(x, edge_index, W1, b1, W2, b2, temp):
    x = np.asarray(x)
    W1_np = np.asarray(W1, dtype=np.float32)
    b1_np = np.asarray(b1, dtype=np.float32)
    W2_np = np.asarray(W2, dtype=np.float32)
    b2_np = np.asarray(b2, dtype=np.float32)
    deg, c, core_data, c4_rank, nc4, slots = _host_prep(edge_index, temp)

    # a guaranteed-zero source row inside each 32768-row window (pad rows)
    zero_row_local = []
    for w in range(NWIN):
        found = None
        for s in range(NCORES):
            r = s * RPS + SHARD + 100
            if r // WIN == w:
                found = r - w * WIN
                break
        assert found is not None
        zero_row_local.append(found)
    win_rows = [min(WIN, TROWS - w * WIN) for w in range(NWIN)]
    sfree = int(c4_rank.max(axis=0).sum()) * 8

    nc = bacc.Bacc("TRN2", target_bir_lowering=False, debug=False,
                   num_devices=NCORES, num_swdge_queues=4)

    xT_d = nc.dram_tensor("xT", [N_FEATS, RPS], F16, kind="ExternalInput")
    W1_d = nc.dram_tensor("W1t", [N_FEATS, HIDDEN], F16, kind="ExternalInput")
    W2_d = nc.dram_tensor("W2t", [HIDDEN, FEAT], F16, kind="ExternalInput")
    b1_d = nc.dram_tensor("b1t", [HIDDEN, 1], F32, kind="ExternalInput")
    b2_d = nc.dram_tensor("b2t", [128, FEAT], F32, kind="ExternalInput")
    dinv_d = nc.dram_tensor("dinvt", [128, NTILES], F32, kind="ExternalInput")
    dinv2_d = nc.dram_tensor("dinv2t", [128, NTILES], F32, kind="ExternalInput")
    sqd_d = nc.dram_tensor("sqdt", [128, NTILES], F32, kind="ExternalInput")
    msk_d = nc.dram_tensor("mskt", [128, NTILES], F32, kind="ExternalInput")
    gidx_d = nc.dram_tensor("gidx", [128, slots // 16], I16, kind="ExternalInput")
    sidx_d = nc.dram_tensor("sidx", [128, sfree], I16, kind="ExternalInput")
    out_d = nc.dram_tensor("outp", [RPS, FEAT], F32, kind="ExternalOutput")

    table = nc.dram_tensor("ttable", [TROWS, STEP], F16,
                       addr_space="Local" if os.environ.get("KERN_LOCAL_TABLE") else "Shared")
    agin = nc.dram_tensor("agin", [RPS, STEP], F16)
    stab = nc.dram_tensor("stab", [RPS, STEP], F16)

    cc = [float(v) for v in c]
    rank_off_nom = np.zeros((NWIN, MAXRANK + 1), dtype=np.int64)
    for w in range(NWIN):
        rank_off_nom[w, 1:] = np.cumsum(c4_rank[w])

    with tile.TileContext(nc) as tc:
        with (
            tc.tile_pool(name="persist", bufs=1) as pp,
            tc.tile_pool(name="psum", bufs=4, space="PSUM") as psp,
        ):
            dinv_t = pp.tile([128, NTILES], F32)
            dinv2_t = pp.tile([128, NTILES], F32)
            sqd_t = pp.tile([128, NTILES], F32)
            msk_t = pp.tile([128, NTILES], F32)
            for tt, dd in ((dinv_t, dinv_d), (dinv2_t, dinv2_d),
                           (sqd_t, sqd_d), (msk_t, msk_d)):
                nc.sync.dma_start(out=tt[:], in_=dd[:])
            acc_t = pp.tile([128, NTILES, FEAT], F32)
            z_t = pp.tile([128, NTILES, FEAT], F32)
            tnext_t = pp.tile([128, NTILES, STEP], F16)
            nc.vector.memset(tnext_t[:], 0)

            # ---------------- MLP ----------------
            with (
                tc.tile_pool(name="mlp", bufs=1) as mp,
                tc.tile_pool(name="mlpw", bufs=3) as mp2,
            ):
                W1_t = mp.tile([128, N_FEATS // 128, HIDDEN], F16)
                for kk in range(N_FEATS // 128):
                    nc.sync.dma_start(out=W1_t[:, kk, :],
                                      in_=W1_d[kk * 128:(kk + 1) * 128, :])
                W2_t = mp.tile([128, HIDDEN // 128, FEAT], F16)
                for kk in range(HIDDEN // 128):
                    nc.sync.dma_start(out=W2_t[:, kk, :],
                                      in_=W2_d[kk * 128:(kk + 1) * 128, :])
                b1_t = mp.tile([128, HIDDEN // 128], F32)
                for kk in range(HIDDEN // 128):
                    nc.sync.dma_start(out=b1_t[:, kk:kk + 1],
                                      in_=b1_d[kk * 128:(kk + 1) * 128, :])
                b2_t = mp.tile([128, FEAT], F32)
                nc.sync.dma_start(out=b2_t[:], in_=b2_d[:])

                hT_t = mp.tile([128, HIDDEN // 128, RPS], F16)
                NT = 512
                for nt in range(RPS // NT):
                    nsl = slice(nt * NT, (nt + 1) * NT)
                    xT_t = mp2.tile([128, N_FEATS // 128, NT], F16, tag="xT")
                    for kk in range(N_FEATS // 128):
                        nc.sync.dma_start(out=xT_t[:, kk, :],
                                          in_=xT_d[kk * 128:(kk + 1) * 128, nsl])
                    for mm in range(HIDDEN // 128):
                        ps = psp.tile([128, NT], F32, tag="hpsum")
                        for kk in range(N_FEATS // 128):
                            nc.tensor.matmul(
                                out=ps[:],
                                lhsT=W1_t[:, kk, mm * 128:(mm + 1) * 128],
                                rhs=xT_t[:, kk, :],
                                start=(kk == 0), stop=(kk == N_FEATS // 128 - 1))
                        nc.scalar.activation(
                            out=hT_t[:, mm, nsl], in_=ps[:],
                            func=mybir.ActivationFunctionType.Relu,
                            bias=b1_t[:, mm:mm + 1], scale=1.0)
                for ti in range(NTILES):
                    tsl = slice(ti * 128, (ti + 1) * 128)
                    ps = psp.tile([128, FEAT], F32, tag="zpsum")
                    for kk in range(HIDDEN // 128):
                        nc.tensor.matmul(out=ps[:], lhsT=hT_t[:, kk, tsl],
                                         rhs=W2_t[:, kk, :],
                                         start=(kk == 0), stop=(kk == 1))
                    nc.vector.tensor_tensor(
                        out=z_t[:, ti, :], in0=ps[:],
                        in1=b2_t[:],
                        op=mybir.AluOpType.add)
                    nc.vector.tensor_tensor(
                        out=tnext_t[:, ti, 0:FEAT], in0=z_t[:, ti, :],
                        in1=dinv_t[:, ti:ti + 1].to_broadcast([128, FEAT]),
                        op=mybir.AluOpType.mult)

            nc.vector.tensor_scalar(
                out=acc_t[:], in0=tnext_t[:, :, 0:FEAT], scalar1=cc[0],
                scalar2=None, op0=mybir.AluOpType.mult)

            # ------------- index planes -------------
            gidx_t = pp.tile([128, slots // 16], I16)
            nc.sync.dma_start(out=gidx_t[:], in_=gidx_d[:])
            sidx_t = pp.tile([128, sfree], I16)
            nc.sync.dma_start(out=sidx_t[:], in_=sidx_d[:])
            zero_t = pp.tile([128, 1280], F16)
            nc.vector.memset(zero_t[:], 0)

            # ------------- propagation -------------
            stack = __import__("contextlib").ExitStack()
            wp = stack.enter_context(tc.tile_pool(name="work", bufs=3))
            p2p = stack.enter_context(tc.tile_pool(name="p2p", bufs=2))
            partp = stack.enter_context(tc.tile_pool(name="partp", bufs=1))
            mcp = stack.enter_context(tc.tile_pool(name="misc", bufs=1))
            nchunks = slots // CHUNK
            cols_per_chunk = CHUNK // (128 * G)
            _maxm = 0
            for _m in range(1, K + 1):
                if abs(cc[_m]) > 1e-300:
                    _maxm = _m
            KI = int(os.environ.get("KERN_ITERS", str(_maxm)))
            SKIP_GS = os.environ.get("KERN_SKIP_GS", "0") == "1"
            SKIP_SC = os.environ.get("KERN_SKIP_SC", "0") == "1"
            for m in range(1, KI + 1):
                nc.sync.dma_start(
                    out=agin[:].rearrange("(t p) s -> p t s", p=128),
                    in_=tnext_t[:])
                if os.environ.get("KERN_LOCAL_TABLE"):
                    for _sh in range(NCORES):
                        nc.sync.dma_start(
                            out=table[_sh * RPS:(_sh + 1) * RPS, :], in_=agin[:])
                else:
                    nc.gpsimd.collective_compute(
                        "AllGather", mybir.AluOpType.bypass,
                        replica_groups=[list(range(NCORES))],
                        ins=[agin[:]], outs=[table[:]])
                for r in range(10):
                    nc.sync.dma_start(out=stab[r * 1280:(r + 1) * 1280, :],
                                      in_=zero_t[:])

                for w in range(NWIN if not SKIP_GS else 0):
                    part_t = partp.tile([128, nc4, FEAT], F16, tag="part")
                    for ci in range(nchunks):
                        g_t = wp.tile([128, CHUNK // 128, FEAT], F16, tag="gt")
                        _emit_dma_gather(
                            nc.gpsimd, g_t[:],
                            table[w * WIN:w * WIN + win_rows[w], 0:FEAT],
                            gidx_t[:, ci * (CHUNK // 16):(ci + 1) * (CHUNK // 16)],
                            CHUNK, elem_size=FEAT, elem_step=STEP,
                            queue_num=int(os.environ.get("KERN_GQ", "1")) and w)
                        p2 = p2p.tile([128, CHUNK // 256, FEAT], F16, tag="p2")
                        nc.vector.tensor_tensor(
                            out=p2[:], in0=g_t[:, 0::2, :], in1=g_t[:, 1::2, :],
                            op=mybir.AluOpType.add)
                        nc.vector.tensor_tensor(
                            out=part_t[:, ci * cols_per_chunk:(ci + 1) * cols_per_chunk, :],
                            in0=p2[:, 0::2, :], in1=p2[:, 1::2, :],
                            op=mybir.AluOpType.add)
                    base = 0
                    sfree_off = 0
                    for k in range(MAXRANK if not SKIP_SC else 0):
                        nck = int(c4_rank[w][k])
                        if nck == 0:
                            continue
                        nc.gpsimd.dma_scatter_add(
                            out_ap=stab[:, 0:FEAT],
                            in_ap=part_t[:, base:base + nck, :],
                            idxs_ap=sidx_t[:, sfree_off:sfree_off + nck * 8],
                            num_idxs=nck * 128, num_idxs_reg=nck * 128,
                            elem_size=FEAT, elem_step=STEP, queue_num=w)
                        base += nck
                        sfree_off += nck * 8

                s_t = mcp.tile([128, NTILES, FEAT], F16, tag="sread")
                nc.sync.dma_start(
                    out=s_t[:],
                    in_=stab[:, 0:FEAT].rearrange("(t p) f -> p t f", p=128))
                nc.vector.tensor_tensor(
                    out=tnext_t[:, :, 0:FEAT], in0=s_t[:],
                    in1=dinv2_t[:].rearrange("p (t o) -> p t o", o=1
                                             ).to_broadcast([128, NTILES, FEAT]),
                    op=mybir.AluOpType.mult)
                tmp_t = mcp.tile([128, NTILES, FEAT], F32, tag="scr")
                nc.vector.tensor_scalar(
                    out=tmp_t[:], in0=tnext_t[:, :, 0:FEAT], scalar1=cc[m],
                    scalar2=None, op0=mybir.AluOpType.mult)
                nc.vector.tensor_tensor(out=acc_t[:], in0=acc_t[:], in1=tmp_t[:],
                                        op=mybir.AluOpType.add)

            # ------------- epilogue -------------
            logit_t = mcp.tile([128, NTILES, FEAT], F32, tag="logit")
            nc.vector.tensor_tensor(
                out=logit_t[:], in0=acc_t[:],
                in1=sqd_t[:].rearrange("p (t o) -> p t o", o=1).to_broadcast(
                    [128, NTILES, FEAT]),
                op=mybir.AluOpType.mult)
            mz_t = mcp.tile([128, NTILES, FEAT], F32, tag="scr")
            nc.vector.tensor_tensor(
                out=mz_t[:], in0=z_t[:],
                in1=msk_t[:].rearrange("p (t o) -> p t o", o=1).to_broadcast(
                    [128, NTILES, FEAT]),
                op=mybir.AluOpType.mult)
            nc.vector.tensor_tensor(out=logit_t[:], in0=logit_t[:], in1=mz_t[:],
                                    op=mybir.AluOpType.add)
            mx_t = mcp.tile([128, NTILES, 1], F32, tag="mx")
            nc.vector.reduce_max(out=mx_t[:], in_=logit_t[:],
                                 axis=mybir.AxisListType.X)
            nc.vector.tensor_tensor(
                out=logit_t[:], in0=logit_t[:],
                in1=mx_t[:].to_broadcast([128, NTILES, FEAT]),
                op=mybir.AluOpType.subtract)
            ex_t = mcp.tile([128, NTILES, FEAT], F32, tag="scr")
            nc.scalar.activation(out=ex_t[:], in_=logit_t[:],
                                 func=mybir.ActivationFunctionType.Exp)
            sm_t = mcp.tile([128, NTILES, 1], F32, tag="sm")
            nc.vector.reduce_sum(out=sm_t[:], in_=ex_t[:],
                                 axis=mybir.AxisListType.X)
            ls_t = mcp.tile([128, NTILES, 1], F32, tag="ls")
            nc.scalar.activation(out=ls_t[:], in_=sm_t[:],
                                 func=mybir.ActivationFunctionType.Ln)
            nc.vector.tensor_tensor(
                out=logit_t[:], in0=logit_t[:],
                in1=ls_t[:].to_broadcast([128, NTILES, FEAT]),
                op=mybir.AluOpType.subtract)
            nc.sync.dma_start(
                out=out_d[:].rearrange("(t p) f -> p t f", p=128),
                in_=logit_t[:])
            stack.close()

    nc.compile()

    deg32 = deg.astype(np.float32)
    dinv32 = np.where(deg32 > 0, 1.0 / np.sqrt(np.maximum(deg32, 1.0)), 0.0
                      ).astype(np.float32)
    in_maps = []
    for cj in range(NCORES):
        sl = slice(cj * SHARD, (cj + 1) * SHARD)
        xs = np.zeros((N_FEATS, RPS), dtype=np.float16)
        xs[:, :SHARD] = np.asarray(x[sl], dtype=np.float32).T.astype(np.float16)
        dv = np.zeros(RPS, np.float32)
        dv[:SHARD] = dinv32[sl]
        dgs = deg32[sl]
        dv2 = np.zeros(RPS, np.float32)
        dv2[:SHARD] = np.where(dgs > 0, 1.0 / np.maximum(dgs, 1.0), 0.0)
        sq = np.zeros(RPS, np.float32)
        sq[:SHARD] = np.sqrt(np.maximum(dgs, 0.0))
        mk = np.zeros(RPS, np.float32)
        mk[:SHARD] = np.where(dgs > 0, 0.0, float(c[0]))
        gplane, splane, _, _ = _build_core_arrays(
            core_data[cj], c4_rank, nc4, slots, zero_row_local)
        im = {
            "xT": xs,
            "W1t": W1_np.astype(np.float16),
            "W2t": W2_np.astype(np.float16),
            "b1t": b1_np.reshape(HIDDEN, 1),
            "b2t": np.tile(b2_np.reshape(1, FEAT), (128, 1)),
            "dinvt": dv.reshape(NTILES, 128).T.copy(),
            "dinv2t": dv2.reshape(NTILES, 128).T.copy(),
            "sqdt": sq.reshape(NTILES, 128).T.copy(),
            "mskt": mk.reshape(NTILES, 128).T.copy(),
            "gidx": gplane,
            "sidx": splane,
        }
        in_maps.append(im)

    import time as _time
    _t0 = _time.time()
    res = run_bass_kernel_spmd(nc, in_maps, core_ids=list(range(NCORES)))
    _dt1 = _time.time() - _t0
    global LAST_EXEC_NS
    LAST_EXEC_NS = getattr(res, "exec_time_ns", None)
    if LAST_EXEC_NS is None and os.environ.get("KERN_TIME"):
        # warm second run: wall time of the execute step (upper bound on HW time)
        _t0 = _time.time()
        res = run_bass_kernel_spmd(nc, in_maps, core_ids=list(range(NCORES)))
        LAST_EXEC_NS = int((_time.time() - _t0) * 1e9)
    outs = [res.results[cj]["outp"][:SHARD] for cj in range(NCORES)]
    return np.concatenate(outs, axis=0).astype(np.float32)

